# revision 1
# baseline (speedup 1.0000x reference)
"""Trainium2 Bass kernel for nn_MlroleNode_64716567216639 (GAT message passing).

Math note: the reference model computes a dense NxN GATv2 attention but only
row 0 of the output (gat_out[0]) feeds the final MLP, so this kernel computes
just that row: e[j,h] = leaky(g_l[j] + g_r[0]) . w_attn, softmax over the 1024
source nodes, then a weighted sum of g_r values, followed by the 3-layer
type-define MLP over the 1023 ambiguous nodes.

Layout: features on partitions, nodes on the free axis (everything transposed
on host). The GAT row-0 computation is replicated on all 8 cores; the final
MLP is sharded 128 nodes per core.
"""
import numpy as np

H = 64
N_AMB = 1023
N = 1024
HEADS = 4
HID = 64
RT = 4
APT = 3
SLOPE = 0.2
NCORES = 8
SHARD = 128  # MLP nodes per core (8*128 = 1024 = N_AMB padded by 1)

_compiled = None  # (nc, static_in_maps_builder)


def _build():
    import concourse.tile as tile
    from concourse import bacc, mybir

    dt = mybir.dt.float32
    AF = mybir.ActivationFunctionType
    ALU = mybir.AluOpType
    AX = mybir.AxisListType

    nc = bacc.Bacc("TRN2", target_bir_lowering=False, debug=False,
                   enable_asserts=False, num_devices=NCORES)

    def din(name, shape):
        return nc.dram_tensor(name, shape, dt, kind="ExternalInput").ap()

    ambT_d = din("ambT", [H, N_AMB])
    hidc_d = din("hidc", [H, 1])
    ta_d = din("ta", [H, RT * APT])
    WselfT_d = din("WselfT", [H, H])
    WmLT_d = din("WmLT", [H, H])
    WmRT_d = din("WmRT", [H, H])
    WtT_d = din("WtT", [H, RT * H])
    btT_d = din("btT", [H, RT])
    bsc_d = din("bsc", [H, 1])
    bmc_d = din("bmc", [H, 1])
    WlT0_d = din("WlT0", [H, 128])
    WlT1_d = din("WlT1", [H, 128])
    WrT_d = din("WrT", [H, HEADS * HID])
    Wexp_d = din("Wexp", [128, 128])
    fold_d = din("fold", [128, H])
    Wd0a_aug_d = din("Wd0a_aug", [H + 1, 64])
    Wd0bT_d = din("Wd0bT", [H, 64])
    Wd1_aug_d = din("Wd1_aug", [65, 128])
    Wd2T_d = din("Wd2T", [128, RT])
    bd2c_d = din("bd2c", [RT, 1])
    mlp_d = din("mlp_cols", [H, SHARD])
    outT_d = nc.dram_tensor("outT", [RT, SHARD], dt, kind="ExternalOutput").ap()

    with tile.TileContext(nc) as tc:
        with tc.tile_pool(name="wp", bufs=1) as wp, \
             tc.tile_pool(name="sb", bufs=1) as sb, \
             tc.tile_pool(name="ps", bufs=1, space="PSUM") as ps:

            # ---- load inputs to SBUF ----
            def load(dram_ap, shape, tag):
                t = wp.tile(shape, dt, tag=tag)
                nc.sync.dma_start(t[:], dram_ap[:])
                return t

            ta_sb = load(ta_d, [H, RT * APT], "ta")
            WselfT = load(WselfT_d, [H, H], "WselfT")
            WmLT = load(WmLT_d, [H, H], "WmLT")
            WmRT = load(WmRT_d, [H, H], "WmRT")
            WtT = load(WtT_d, [H, RT * H], "WtT")
            btT = load(btT_d, [H, RT], "btT")
            bsc = load(bsc_d, [H, 1], "bsc")
            bmc = load(bmc_d, [H, 1], "bmc")
            hidc = load(hidc_d, [H, 1], "hidc")
            WlT = [load(WlT0_d, [H, 128], "WlT0"), load(WlT1_d, [H, 128], "WlT1")]
            WrT = load(WrT_d, [H, HEADS * HID], "WrT")
            Wexp = load(Wexp_d, [128, 128], "Wexp")
            fold = load(fold_d, [128, H], "fold")
            Wd0a_aug = load(Wd0a_aug_d, [H + 1, 64], "Wd0a")
            Wd0bT = load(Wd0bT_d, [H, 64], "Wd0b")
            Wd1_aug = load(Wd1_aug_d, [65, 128], "Wd1")
            Wd2T = load(Wd2T_d, [128, RT], "Wd2")
            bd2c = load(bd2c_d, [RT, 1], "bd2c")

            hT = wp.tile([H, N], dt, tag="hT")
            nc.sync.dma_start(hT[:, 1:N], ambT_d[:])
            mlp_aug = wp.tile([H + 1, SHARD], dt, tag="mlpa")
            nc.sync.dma_start(mlp_aug[0:H, :], mlp_d[:])
            nc.vector.memset(mlp_aug[H:H + 1, :], 1.0)
            # preload ACT tables (Exp/Sigmoid) off the critical softmax path
            warm = wp.tile([1, 4], dt, tag="warm")
            nc.vector.memset(warm[:], 0.0)
            warm_act = wp.tile([1, 4], dt, tag="warmact")
            nc.scalar.activation(warm_act[0:1, 0:1], warm[0:1, 0:1], AF.Exp)

            def leaky(out_ap, in_ap):
                # in_ap must be SBUF (stt can read at most one PSUM input)
                nc.vector.scalar_tensor_tensor(out=out_ap, in0=in_ap, scalar=SLOPE,
                                               in1=in_ap, op0=ALU.mult, op1=ALU.max)

            def leaky_psum(out_ap, psum_ap, scratch_ap):
                # leaky(x) = max(0.2*x, x) with x in PSUM: two DVE ops
                nc.vector.tensor_scalar_mul(scratch_ap, psum_ap, SLOPE)
                nc.vector.tensor_tensor(out_ap, scratch_ap, psum_ap, op=ALU.max)

            # ---- prologue: role-type routing + merge chain -> h1 [64,1] ----
            tsum = sb.tile([H, RT], dt, tag="tsum")
            nc.vector.reduce_sum(tsum[:], ta_sb[:].rearrange("p (t a) -> p t a", a=APT),
                                 axis=AX.X)
            tmean = sb.tile([H, RT], dt, tag="tmean")
            nc.vector.tensor_scalar_mul(tmean[:], tsum[:], 1.0 / APT)
            tmp_ps = ps.tile([H, RT], dt, tag="sp", bufs=1)
            for t in range(RT):
                nc.tensor.matmul(tmp_ps[:, t:t + 1], WtT[:, H * t:H * (t + 1)],
                                 tmean[:, t:t + 1], start=True, stop=True)
            tmpc = sb.tile([H, RT], dt, tag="tmpc")
            nc.vector.tensor_add(tmpc[:], tmp_ps[:], btT[:])
            C_ps = ps.tile([H, RT], dt, tag="sp", bufs=1)
            nc.tensor.matmul(C_ps[:], WmRT[:], tmpc[:], start=True, stop=True)
            C_sb = sb.tile([H, RT], dt, tag="C")
            nc.vector.tensor_scalar_add(C_sb[:], C_ps[:], bmc[:])

            h1_ps = ps.tile([H, 1], dt, tag="sp", bufs=1)
            nc.tensor.matmul(h1_ps[:], WselfT[:], hidc[:], start=True, stop=True)
            h1 = sb.tile([H, 1], dt, tag="h1", bufs=2)
            nc.vector.tensor_scalar_add(h1[:], h1_ps[:], bsc[:])
            for t in range(RT):
                hp = ps.tile([H, 1], dt, tag="sp", bufs=1)
                nc.tensor.matmul(hp[:], WmLT[:], h1[:], start=True, stop=True)
                u = sb.tile([H, 1], dt, tag="u", bufs=2)
                nc.vector.tensor_scalar_add(u[:], hp[:], C_sb[:, t:t + 1])
                h1n = sb.tile([H, 1], dt, tag="h1", bufs=2)
                leaky(h1n[:], u[:])
                h1 = h1n
            nc.vector.tensor_copy(hT[:, 0:1], h1[:])

            # ---- GAT row 0, two head-pair blocks ----
            h2_ps = ps.tile([H, 1], dt, tag="h2ps", bufs=1)
            for b in range(2):
                # g_r0 column for this head-pair block (attention query side)
                gr0_ps = ps.tile([128, 1], dt, tag="sp", bufs=1)
                nc.tensor.matmul(gr0_ps[:], WrT[:, 128 * b:128 * b + 128], h1[:],
                                 start=True, stop=True)
                gr0c = sb.tile([128, 1], dt, tag="gr0", bufs=2)
                nc.vector.tensor_copy(gr0c[:], gr0_ps[:])
                gl_ps = ps.tile([128, N], dt, tag="gle", bufs=2)
                for c in (0, 512):
                    nc.tensor.matmul(gl_ps[:, c:c + 512], WlT[b][:], hT[:, c:c + 512],
                                     start=True, stop=True)
                t_sb = sb.tile([128, N], dt, tag="t", bufs=2)
                u_sb = sb.tile([128, N], dt, tag="scr", bufs=2)
                nc.scalar.activation(u_sb[:], gl_ps[:], AF.Identity, bias=gr0c[:])
                leaky(t_sb[:], u_sb[:])
                gr_ps = ps.tile([128, N], dt, tag="gr", bufs=1)
                for c in (0, 512):
                    nc.tensor.matmul(gr_ps[:, c:c + 512],
                                     WrT[:, 128 * b:128 * b + 128],
                                     hT[:, c:c + 512], start=True, stop=True)
                e_ps = ps.tile([128, N], dt, tag="gle", bufs=2)
                for c in (0, 512):
                    nc.tensor.matmul(e_ps[:, c:c + 512], Wexp[:], t_sb[:, c:c + 512],
                                     start=True, stop=True)
                # softmax over the 1024 source nodes (per head, replicated x64).
                # logits are O(5) so no max subtraction is needed in fp32.
                pexp = sb.tile([128, N], dt, tag="pexp", bufs=2)
                ssum = sb.tile([128, 1], dt, tag="s", bufs=4)
                nc.scalar.activation(pexp[:], e_ps[:], AF.Exp, bias=0.0,
                                     accum_out=ssum[:])
                # weighted value sum over source nodes (fused mul + row-sum)
                scr = sb.tile([128, N], dt, tag="scr", bufs=2)
                att_u = sb.tile([128, 1], dt, tag="acc", bufs=4)
                nc.vector.scalar_tensor_tensor(
                    out=scr[:], in0=pexp[:], scalar=1.0, in1=gr_ps[:],
                    op0=ALU.mult, op1=ALU.mult, accum_out=att_u[:])
                rs = sb.tile([128, 1], dt, tag="s", bufs=4)
                nc.vector.reciprocal(rs[:], ssum[:])
                att_n = sb.tile([128, 1], dt, tag="acc", bufs=4)
                nc.vector.tensor_mul(att_n[:], att_u[:], rs[:])
                # fold heads: h2 += 0.25 * sum over the 2 heads in this block
                nc.tensor.matmul(h2_ps[:], fold[:], att_n[:], start=(b == 0),
                                 stop=(b == 1))

            h2 = sb.tile([H, 1], dt, tag="h2")
            nc.vector.tensor_copy(h2[:], h2_ps[:])

            # ---- final MLP on this core's 128-node shard ----
            c0_ps = ps.tile([H, 1], dt, tag="sp", bufs=1)
            nc.tensor.matmul(c0_ps[:], Wd0bT[:], h2[:], start=True, stop=True)
            c0col = sb.tile([H, 1], dt, tag="c0")
            nc.vector.tensor_copy(c0col[:], c0_ps[:])
            y0_ps = ps.tile([64, SHARD], dt, tag="sp", bufs=1)
            nc.tensor.matmul(y0_ps[:], Wd0a_aug[:], mlp_aug[:], start=True, stop=True)
            y0_aug = sb.tile([65, SHARD], dt, tag="y0")
            nc.vector.memset(y0_aug[64:65, :], 1.0)
            y0u = sb.tile([64, SHARD], dt, tag="yscr", bufs=2)
            nc.scalar.activation(y0u[:], y0_ps[:], AF.Identity, bias=c0col[:])
            leaky(y0_aug[0:64, :], y0u[:])
            y1_ps = ps.tile([128, SHARD], dt, tag="sp", bufs=1)
            nc.tensor.matmul(y1_ps[:], Wd1_aug[:], y0_aug[:], start=True, stop=True)
            y1 = sb.tile([128, SHARD], dt, tag="y1")
            y1scr = sb.tile([128, SHARD], dt, tag="yscr", bufs=2)
            leaky_psum(y1[:], y1_ps[:], y1scr[:])
            o_ps = ps.tile([RT, SHARD], dt, tag="sp", bufs=1)
            nc.tensor.matmul(o_ps[:], Wd2T[:], y1[:], start=True, stop=True)
            # sigmoid(z) = 1/(1+exp(-z)) using the already-loaded Exp table
            # (avoids a 1.3us Sigmoid ACT-table load on the critical path)
            o_e = sb.tile([RT, SHARD], dt, tag="oe")
            nc.scalar.activation(o_e[:], o_ps[:], AF.Exp, bias=bd2c[:], scale=-1.0)
            o_1p = sb.tile([RT, SHARD], dt, tag="o1p")
            nc.vector.tensor_scalar_add(o_1p[:], o_e[:], 1.0)
            o_sb = sb.tile([RT, SHARD], dt, tag="o")
            nc.vector.reciprocal(o_sb[:], o_1p[:])
            nc.sync.dma_start(outT_d[:], o_sb[:])

    nc.compile()
    return nc


def _prep_inputs(inputs):
    f32 = np.float32

    def c(a):
        return np.ascontiguousarray(a, dtype=f32)

    hidden = np.asarray(inputs["hidden"], f32)
    ambiguous = np.asarray(inputs["ambiguous"], f32)
    type_agents = np.asarray(inputs["type_agents"], f32)
    W_self = np.asarray(inputs["W_self"], f32)
    b_self = np.asarray(inputs["b_self"], f32)
    W_merge = np.asarray(inputs["W_merge"], f32)
    b_merge = np.asarray(inputs["b_merge"], f32)
    W_trans = np.asarray(inputs["W_trans"], f32)
    b_trans = np.asarray(inputs["b_trans"], f32)
    W_l = np.asarray(inputs["W_l"], f32)
    W_r = np.asarray(inputs["W_r"], f32)
    w_attn = np.asarray(inputs["w_attn"], f32)
    Wd0 = np.asarray(inputs["Wd0"], f32)
    bd0 = np.asarray(inputs["bd0"], f32)
    Wd1 = np.asarray(inputs["Wd1"], f32)
    bd1 = np.asarray(inputs["bd1"], f32)
    Wd2 = np.asarray(inputs["Wd2"], f32)
    bd2 = np.asarray(inputs["bd2"], f32)

    ambT = c(ambiguous.T)                                   # [64, 1023]
    WlT_full = c(W_l.T)                                     # [64, 256]
    Wexp = np.zeros((128, 128), f32)
    for hh in range(2):
        Wexp[hh * 64:(hh + 1) * 64, hh * 64:(hh + 1) * 64] = w_attn[:, None]
    fold = np.zeros((128, 64), f32)
    fold[np.arange(128), np.arange(128) % 64] = 0.25

    shared = {
        "ambT": ambT,
        "hidc": c(hidden.reshape(H, 1)),
        "ta": c(type_agents.reshape(RT * APT, H).T),
        "WselfT": c(W_self.T),
        "WmLT": c(W_merge[:, :H].T),
        "WmRT": c(W_merge[:, H:].T),
        "WtT": c(np.concatenate([W_trans[t].T for t in range(RT)], axis=1)),
        "btT": c(b_trans.T),
        "bsc": c(b_self.reshape(H, 1)),
        "bmc": c(b_merge.reshape(H, 1)),
        "WlT0": c(WlT_full[:, :128]),
        "WlT1": c(WlT_full[:, 128:]),
        "WrT": c(W_r.T),
        "Wexp": Wexp,
        "fold": fold,
        "Wd0a_aug": c(np.vstack([Wd0[:, :H].T, bd0[None, :]])),
        "Wd0bT": c(Wd0[:, H:].T),
        "Wd1_aug": c(np.vstack([Wd1.T, bd1[None, :]])),
        "Wd2T": c(Wd2.T),
        # negated: used as the bias of Exp(-z) inside the exp-based sigmoid
        "bd2c": c(-bd2.reshape(RT, 1)),
    }
    amb_pad = np.zeros((H, NCORES * SHARD), f32)
    amb_pad[:, :N_AMB] = ambT
    in_maps = []
    for cidx in range(NCORES):
        m = dict(shared)
        m["mlp_cols"] = c(amb_pad[:, cidx * SHARD:(cidx + 1) * SHARD])
        in_maps.append(m)
    return in_maps


def kernel(**inputs) -> np.ndarray:
    global _compiled
    if _compiled is None:
        _compiled = _build()
    nc = _compiled
    from concourse import bass_utils

    in_maps = _prep_inputs(inputs)
    res = bass_utils.run_bass_kernel_spmd(nc, in_maps, core_ids=list(range(NCORES)))
    out = np.empty((N_AMB, RT), np.float32)
    for cidx in range(NCORES):
        lo = cidx * SHARD
        hi = min(lo + SHARD, N_AMB)
        out[lo:hi, :] = res.results[cidx]["outT"][:, :hi - lo].T
    return out



# revision 4
# speedup vs baseline: 1.7114x; 1.7114x over previous
"""Trainium2 Bass kernel for nn_MlroleNode_64716567216639 (GAT message passing).

Only row 0 of the NxN GATv2 attention feeds the output, so the kernel computes
just that row. All matmuls run in bf16 (fp32 matmul on TRN2 is a 2-pass
LOW/HIGH split at ~4x the cost); accumulation stays fp32 in PSUM. Inputs are
packed host-side into 3 DRAM buffers (one fp32, two bf16) so the load is 3
DMAs instead of 22. Bias+LeakyReLU is fused into single ACT ops (Prelu with
alpha=0.2, same table set as Exp), sigmoid is computed via tanh (same set).
The GAT leaky passes are split between ACT (1 op) and DVE (2 ops) to balance
engines. h1 (node 0) is written into a reserved bf16 column adjacent to the
ambiguous columns so the pairwise job is two clean 512-col chunks per block.

Layout: features on partitions, nodes on the free axis. GAT row-0 work is
replicated on all 8 cores; the final MLP is sharded 128 nodes per core.
"""
import numpy as np

H = 64
N_AMB = 1023
HEADS = 4
HID = 64
RT = 4
APT = 3
SLOPE = 0.2
NCORES = 8
SHARD = 128

# packA (fp32) column layout
A_TA = 0        # 12 cols: type_agents [64, RT*APT]
A_BTT = 12      # 4 cols: b_trans.T
A_BSC = 16      # 1 col
A_BMC = 17      # 1 col
A_BD2H = 18     # 1 col: 0.5*bd2 in rows 0..3
A_COLS = 19

# packB (bf16) column layout
B_HID = 0
B_WSELF = 1
B_WML = 65
B_WMR = 129
B_WTT = 193     # 256 cols
B_WL = 449      # 256 cols (two 128-col head-pair blocks)
B_WR = 705      # 256 cols
B_WD0B = 961    # 64 cols
B_H1 = 1025     # 1 col: h1 slot, written on device
B_AMB = 1026    # 1023 cols
B_COLS = 2049

# packC (bf16) column layout
C_WEXP = 0      # 128
C_FOLD = 128    # 64
C_WD1 = 192     # 128 (rows 0..64)
C_WD2 = 320     # 4
C_WD0A = 324    # 64 (rows 0..64)
C_MLP = 388     # 128 (rows 0..64, row 64 = ones)
C_COLS = 516

_compiled = None


def _build():
    import concourse.tile as tile
    from concourse import bacc, mybir

    f32 = mybir.dt.float32
    bf16 = mybir.dt.bfloat16
    AF = mybir.ActivationFunctionType
    ALU = mybir.AluOpType
    AX = mybir.AxisListType

    nc = bacc.Bacc("TRN2", target_bir_lowering=False, debug=False,
                   enable_asserts=False, num_devices=NCORES)

    pa_d = nc.dram_tensor("packA", [H, A_COLS], f32, kind="ExternalInput").ap()
    pb_d = nc.dram_tensor("packB", [H, B_COLS], bf16, kind="ExternalInput").ap()
    pc_d = nc.dram_tensor("packC", [128, C_COLS], bf16, kind="ExternalInput").ap()
    outT_d = nc.dram_tensor("outT", [RT, SHARD], f32, kind="ExternalOutput").ap()

    with tile.TileContext(nc) as tc:
        with tc.tile_pool(name="wp", bufs=1) as wp, \
             tc.tile_pool(name="sb", bufs=1) as sb, \
             tc.tile_pool(name="ps", bufs=1, space="PSUM") as ps:

            # ---- ACT table warm (Exp/Prelu/Tanh share exp_and_others) ----
            warm = wp.tile([1, 4], f32, tag="warm")
            nc.vector.memset(warm[:], 0.0)
            warm_o = wp.tile([1, 4], f32, tag="warmo")
            nc.scalar.activation(warm_o[0:1, 0:1], warm[0:1, 0:1], AF.Exp)

            # ---- input loads: 3 packed DMAs ----
            pB = wp.tile([H, B_COLS], bf16, tag="pB")
            nc.sync.dma_start(pB[:], pb_d[:])
            pA = wp.tile([H, A_COLS], f32, tag="pA")
            nc.sync.dma_start(pA[:], pa_d[:])
            pC = wp.tile([128, C_COLS], bf16, tag="pC")
            nc.sync.dma_start(pC[:], pc_d[:])

            y0_aug = sb.tile([65, SHARD], bf16, tag="y0a")
            nc.vector.memset(y0_aug[64:65, :], 1.0)

            WlT = [pB[0:H, B_WL + 128 * b:B_WL + 128 * (b + 1)] for b in (0, 1)]
            WrT = [pB[0:H, B_WR + 128 * b:B_WR + 128 * (b + 1)] for b in (0, 1)]
            Wexp = pC[0:128, C_WEXP:C_WEXP + 128]
            # rhs chunks: c0 includes the h1 column (B_H1) + amb cols 0..510
            rhs_c = [pB[0:H, B_H1:B_H1 + 512], pB[0:H, B_H1 + 512:B_H1 + 1024]]

            # ---- prologue: role means + merge chain -> h1 (bf16, col B_H1) ----
            tsum = sb.tile([H, RT], f32, tag="tsum")
            nc.vector.reduce_sum(
                tsum[:], pA[0:H, A_TA:A_TA + RT * APT].rearrange(
                    "p (t a) -> p t a", a=APT), axis=AX.X)
            tmean = sb.tile([H, RT], bf16, tag="tmean")
            nc.vector.tensor_scalar_mul(tmean[:], tsum[:], 1.0 / APT)

            tmp_ps = ps.tile([H, RT], f32, tag="sp", bufs=1)
            for t in range(RT):
                nc.tensor.matmul(tmp_ps[:, t:t + 1],
                                 pB[0:H, B_WTT + H * t:B_WTT + H * (t + 1)],
                                 tmean[:, t:t + 1], start=True, stop=True)
            tmpc = sb.tile([H, RT], bf16, tag="tmpc")
            nc.vector.tensor_tensor(tmpc[:], tmp_ps[:], pA[0:H, A_BTT:A_BTT + RT],
                                    op=ALU.add)
            C_ps = ps.tile([H, RT], f32, tag="sp", bufs=1)
            nc.tensor.matmul(C_ps[:], pB[0:H, B_WMR:B_WMR + H], tmpc[:],
                             start=True, stop=True)
            C_sb = sb.tile([H, RT], f32, tag="C")
            nc.vector.tensor_scalar_add(C_sb[:], C_ps[:], pA[0:H, A_BMC:A_BMC + 1])

            h1_ps = ps.tile([H, 1], f32, tag="sp", bufs=1)
            nc.tensor.matmul(h1_ps[:], pB[0:H, B_WSELF:B_WSELF + H],
                             pB[0:H, B_HID:B_HID + 1], start=True, stop=True)
            h1 = sb.tile([H, 1], bf16, tag="h1", bufs=2)
            nc.vector.tensor_scalar_add(h1[:], h1_ps[:], pA[0:H, A_BSC:A_BSC + 1])

            # big GAT matmuls that do not depend on h1 (chunk c1), interleaved
            # into the PE gaps of the serial merge chain
            gl_ps = {}
            gr_ps = {}
            early = [("gl", 1, 0), ("gl", 1, 1), ("gr", 1, 0), ("gr", 1, 1)]
            for t in range(RT):
                hp = ps.tile([H, 1], f32, tag="sp", bufs=1)
                nc.tensor.matmul(hp[:], pB[0:H, B_WML:B_WML + H], h1[:],
                                 start=True, stop=True)
                kind, c, b = early[t]
                pool_tag = "big" if kind == "gl" else "gr"
                bufs = 4 if kind == "gl" else 2
                dst = ps.tile([128, 512], f32, tag=pool_tag, bufs=bufs)
                W = WlT[b] if kind == "gl" else WrT[b]
                nc.tensor.matmul(dst[:], W, rhs_c[c], start=True, stop=True)
                (gl_ps if kind == "gl" else gr_ps)[(c, b)] = dst
                if t < RT - 1:
                    h1n = sb.tile([H, 1], bf16, tag="h1", bufs=2)
                    nc.scalar.activation(h1n[:], hp[:], AF.Prelu,
                                         bias=C_sb[:, t:t + 1], alpha=SLOPE)
                    h1 = h1n
                else:
                    # final h1 goes straight into the packB h1 column
                    nc.scalar.activation(pB[0:H, B_H1:B_H1 + 1], hp[:], AF.Prelu,
                                         bias=C_sb[:, t:t + 1], alpha=SLOPE)

            h1col = pB[0:H, B_H1:B_H1 + 1]

            # ---- attention query side: gr0 column per block ----
            gr0c = []
            for b in (0, 1):
                g_ps = ps.tile([128, 1], f32, tag="sp", bufs=1)
                nc.tensor.matmul(g_ps[:], WrT[b], h1col, start=True, stop=True)
                g_sb = sb.tile([128, 1], f32, tag="gr0", bufs=2)
                nc.vector.tensor_copy(g_sb[:], g_ps[:])
                gr0c.append(g_sb)

            # remaining h1-dependent big matmuls (chunk c0 holds the h1 col)
            for b in (0, 1):
                dst = ps.tile([128, 512], f32, tag="big", bufs=4)
                nc.tensor.matmul(dst[:], WlT[b], rhs_c[0], start=True, stop=True)
                gl_ps[(0, b)] = dst

            # ---- leaky(gl + gr0): ACT for b=0 chunks, DVE (2 ops) for b=1 ----
            t_sb = {}

            def leaky_act(c, b):
                t_t = sb.tile([128, 512], bf16, tag="t", bufs=4)
                nc.scalar.activation(t_t[:], gl_ps[(c, b)][:], AF.Prelu,
                                     bias=gr0c[b][:], alpha=SLOPE)
                t_sb[(c, b)] = t_t

            def leaky_dve(c, b):
                u = sb.tile([128, 512], f32, tag="u", bufs=2)
                nc.vector.tensor_scalar(u[:], gl_ps[(c, b)][:], gr0c[b][:], SLOPE,
                                        op0=ALU.add, op1=ALU.mult)
                t_t = sb.tile([128, 512], bf16, tag="t", bufs=4)
                nc.vector.scalar_tensor_tensor(out=t_t[:], in0=gl_ps[(c, b)][:],
                                               scalar=gr0c[b][:], in1=u[:],
                                               op0=ALU.add, op1=ALU.max)
                t_sb[(c, b)] = t_t

            # ---- softmax numerator/denominator machinery per (chunk, block) ----
            pexp = {}
            ssum = {}
            att_u = {}

            def exp_chunk(c, b):
                e = ps.tile([128, 512], f32, tag="big", bufs=4)
                nc.tensor.matmul(e[:], Wexp, t_sb[(c, b)][:], start=True, stop=True)
                p = sb.tile([128, 512], f32, tag="pexp", bufs=4)
                s = sb.tile([128, 1], f32, tag="s", bufs=4)
                nc.scalar.activation(p[:], e[:], AF.Exp, bias=0.0, accum_out=s[:])
                pexp[(c, b)] = p
                ssum[(c, b)] = s

            def wsum_chunk(c, b):
                scr = sb.tile([128, 512], bf16, tag="scr", bufs=2)
                a = sb.tile([128, 1], f32, tag="au", bufs=4)
                nc.vector.scalar_tensor_tensor(
                    out=scr[:], in0=pexp[(c, b)][:], scalar=1.0,
                    in1=gr_ps[(c, b)][:], op0=ALU.mult, op1=ALU.mult,
                    accum_out=a[:])
                att_u[(c, b)] = a

            # pipeline: chunk c1 first (its gl/gr are precomputed)
            leaky_act(1, 0)
            leaky_dve(1, 1)
            exp_chunk(1, 0)
            exp_chunk(1, 1)
            leaky_act(0, 0)
            leaky_dve(0, 1)
            wsum_chunk(1, 0)
            wsum_chunk(1, 1)
            # gr chunk c0 (reuses the c1 psum slots once wsum c1 is done)
            for b in (0, 1):
                dst = ps.tile([128, 512], f32, tag="gr", bufs=2)
                nc.tensor.matmul(dst[:], WrT[b], rhs_c[0], start=True, stop=True)
                gr_ps[(0, b)] = dst
            exp_chunk(0, 0)
            exp_chunk(0, 1)
            wsum_chunk(0, 0)
            wsum_chunk(0, 1)

            # ---- combine chunks, normalize, fold heads ----
            h2_ps = ps.tile([H, 1], f32, tag="h2ps", bufs=1)
            for b in (0, 1):
                den = sb.tile([128, 1], f32, tag="den", bufs=2)
                nc.vector.tensor_tensor(den[:], ssum[(0, b)][:], ssum[(1, b)][:],
                                        op=ALU.add)
                num = sb.tile([128, 1], f32, tag="num", bufs=2)
                nc.vector.tensor_tensor(num[:], att_u[(0, b)][:], att_u[(1, b)][:],
                                        op=ALU.add)
                rs = sb.tile([128, 1], f32, tag="rs", bufs=2)
                nc.vector.reciprocal(rs[:], den[:])
                att_n = sb.tile([128, 1], bf16, tag="an", bufs=2)
                nc.vector.tensor_tensor(att_n[:], num[:], rs[:], op=ALU.mult)
                nc.tensor.matmul(h2_ps[:], pC[0:128, C_FOLD:C_FOLD + H], att_n[:],
                                 start=(b == 0), stop=(b == 1))

            # ---- final MLP on this core's 128-node shard ----
            h2 = sb.tile([H, 1], bf16, tag="h2")
            nc.vector.tensor_copy(h2[:], h2_ps[:])
            c0_ps = ps.tile([H, 1], f32, tag="sp", bufs=1)
            nc.tensor.matmul(c0_ps[:], pB[0:H, B_WD0B:B_WD0B + H], h2[:],
                             start=True, stop=True)
            c0col = sb.tile([H, 1], f32, tag="c0")
            nc.vector.tensor_copy(c0col[:], c0_ps[:])
            y0_ps = ps.tile([H, SHARD], f32, tag="big", bufs=4)
            nc.tensor.matmul(y0_ps[:], pC[0:65, C_WD0A:C_WD0A + H],
                             pC[0:65, C_MLP:C_MLP + SHARD], start=True, stop=True)
            nc.scalar.activation(y0_aug[0:H, :], y0_ps[:], AF.Prelu,
                                 bias=c0col[:], alpha=SLOPE)
            y1_ps = ps.tile([128, SHARD], f32, tag="big", bufs=4)
            nc.tensor.matmul(y1_ps[:], pC[0:65, C_WD1:C_WD1 + 128], y0_aug[:],
                             start=True, stop=True)
            y1 = sb.tile([128, SHARD], bf16, tag="y1")
            nc.scalar.activation(y1[:], y1_ps[:], AF.Prelu, bias=0.0, alpha=SLOPE)
            o_ps = ps.tile([RT, SHARD], f32, tag="sp", bufs=1)
            nc.tensor.matmul(o_ps[:], pC[0:128, C_WD2:C_WD2 + RT], y1[:],
                             start=True, stop=True)
            # sigmoid(z) = 0.5 + 0.5*tanh(0.5*z + 0.5*bd2); tanh shares the
            # already-loaded exp table set
            th = sb.tile([RT, SHARD], f32, tag="th")
            nc.scalar.activation(th[:], o_ps[:], AF.Tanh,
                                 bias=pA[0:RT, A_BD2H:A_BD2H + 1], scale=0.5)
            o_sb = sb.tile([RT, SHARD], f32, tag="o")
            nc.vector.tensor_scalar(o_sb[:], th[:], 0.5, 0.5,
                                    op0=ALU.mult, op1=ALU.add)
            nc.sync.dma_start(outT_d[:], o_sb[:])

    nc.compile()
    return nc


def _prep_inputs(inputs):
    import ml_dtypes
    f32 = np.float32
    bf = ml_dtypes.bfloat16

    def c(a):
        return np.ascontiguousarray(a, dtype=f32)

    hidden = np.asarray(inputs["hidden"], f32)
    ambiguous = np.asarray(inputs["ambiguous"], f32)
    type_agents = np.asarray(inputs["type_agents"], f32)
    W_self = np.asarray(inputs["W_self"], f32)
    b_self = np.asarray(inputs["b_self"], f32)
    W_merge = np.asarray(inputs["W_merge"], f32)
    b_merge = np.asarray(inputs["b_merge"], f32)
    W_trans = np.asarray(inputs["W_trans"], f32)
    b_trans = np.asarray(inputs["b_trans"], f32)
    W_l = np.asarray(inputs["W_l"], f32)
    W_r = np.asarray(inputs["W_r"], f32)
    w_attn = np.asarray(inputs["w_attn"], f32)
    Wd0 = np.asarray(inputs["Wd0"], f32)
    bd0 = np.asarray(inputs["bd0"], f32)
    Wd1 = np.asarray(inputs["Wd1"], f32)
    bd1 = np.asarray(inputs["bd1"], f32)
    Wd2 = np.asarray(inputs["Wd2"], f32)
    bd2 = np.asarray(inputs["bd2"], f32)

    packA = np.zeros((H, A_COLS), f32)
    packA[:, A_TA:A_TA + RT * APT] = type_agents.reshape(RT * APT, H).T
    packA[:, A_BTT:A_BTT + RT] = b_trans.T
    packA[:, A_BSC] = b_self
    packA[:, A_BMC] = b_merge
    packA[0:RT, A_BD2H] = 0.5 * bd2

    packB = np.zeros((H, B_COLS), f32)
    packB[:, B_HID] = hidden[0]
    packB[:, B_WSELF:B_WSELF + H] = W_self.T
    packB[:, B_WML:B_WML + H] = W_merge[:, :H].T
    packB[:, B_WMR:B_WMR + H] = W_merge[:, H:].T
    packB[:, B_WTT:B_WTT + RT * H] = np.concatenate(
        [W_trans[t].T for t in range(RT)], axis=1)
    packB[:, B_WL:B_WL + 2 * 128] = W_l.T
    packB[:, B_WR:B_WR + 2 * 128] = W_r.T
    packB[:, B_WD0B:B_WD0B + H] = Wd0[:, H:].T
    packB[:, B_AMB:B_AMB + N_AMB] = ambiguous.T
    packB16 = packB.astype(bf)

    Wexp = np.zeros((128, 128), f32)
    for hh in range(2):
        Wexp[hh * 64:(hh + 1) * 64, hh * 64:(hh + 1) * 64] = w_attn[:, None]
    fold = np.zeros((128, H), f32)
    fold[np.arange(128), np.arange(128) % H] = 0.25

    packC = np.zeros((128, C_COLS), f32)
    packC[:, C_WEXP:C_WEXP + 128] = Wexp
    packC[:, C_FOLD:C_FOLD + H] = fold
    packC[0:65, C_WD1:C_WD1 + 128] = np.vstack([Wd1.T, bd1[None, :]])
    packC[:, C_WD2:C_WD2 + RT] = Wd2.T
    packC[0:65, C_WD0A:C_WD0A + H] = np.vstack([Wd0[:, :H].T, bd0[None, :]])
    packC[64, C_MLP:C_MLP + SHARD] = 1.0

    amb_pad = np.zeros((H, NCORES * SHARD), f32)
    amb_pad[:, :N_AMB] = ambiguous.T
    in_maps = []
    for cidx in range(NCORES):
        pc = packC.copy()
        pc[0:H, C_MLP:C_MLP + SHARD] = amb_pad[:, cidx * SHARD:(cidx + 1) * SHARD]
        in_maps.append({
            "packA": packA,
            "packB": packB16,
            "packC": pc.astype(bf),
        })
    return in_maps


def kernel(**inputs) -> np.ndarray:
    global _compiled
    if _compiled is None:
        _compiled = _build()
    nc = _compiled
    from concourse import bass_utils

    in_maps = _prep_inputs(inputs)
    res = bass_utils.run_bass_kernel_spmd(nc, in_maps, core_ids=list(range(NCORES)))
    out = np.empty((N_AMB, RT), np.float32)
    for cidx in range(NCORES):
        lo = cidx * SHARD
        hi = min(lo + SHARD, N_AMB)
        out[lo:hi, :] = res.results[cidx]["outT"][:, :hi - lo].T
    return out
